# revision 1
# baseline (speedup 1.0000x reference)
"""BlockAttentionResidual Trainium2 kernel.

Math (per token t, feature dim D=1024, over N+1=9 blocks):
    ssq[n,t]  = sum_d v[n,t,d]^2
    rq[n,t]   = (ssq/D + eps)^(-1/2)        (computed as exp(-0.5*ln(ssq/D+eps)))
    logit     = (sum_d w2[d]*v[n,t,d]) * rq      where w2 = proj_w*norm_w
    w[n,t]    = softmax over n of logit
    h[t,d]    = sum_n w[n,t] * v[n,t,d]

Sharding: B*T = 8192 tokens split evenly across 8 cores (1024 tokens/core).

Host-side prep: per core the 9 blocks are pre-interleaved into
vstack[quad, p, (g,d)] where partition p = 14*n + t' stacks the 9 blocks of
14 tokens (126 rows) and the free dim holds 4 such token-groups (one PSUM
page worth = 56 tokens per "quad"). This makes each quad's input a single
contiguous [126, 4096] DMA with 16KB-per-partition descriptors.

Per-quad on-chip pipeline:
  - ssq:  ScalarE activation(Square) with accum_out       (1 pass)
  - dot:  VectorE scalar_tensor_tensor mult/mult accum    (1 pass)
  - softmax over n: TensorE matmuls against a 0/1 mask M[p,t'] = (p%14==t')
      Z = M^T @ exp(logits), and M @ (1/Z) broadcasts 1/Z back to rows.
  - h:    TensorE matmul  h[t',d] = sum_p lhsT[p,t'] * v[p,d]
      with lhsT = M * w_col, 4 groups packed into one [128,1024] PSUM page
      at partition offsets 0/32/64/96 (PE column-group tiling).
  - PSUM -> SBUF copy split between VectorE and ScalarE, then DMA out.
"""

import os
import sys
import numpy as np

for _p in ("/opt/trn_rl_repo", "/root/.axon_site/_ro/trn_rl_repo"):
    if os.path.isdir(_p) and _p not in sys.path:
        sys.path.append(_p)

N_CORES = 8
N, B, T, D = 8, 4, 2048, 1024
EPS = 1e-6
TOK = (B * T) // N_CORES          # 1024 tokens per core
NB = N + 1                        # 9 stacked blocks
GROUP = 14                        # tokens per group (14*9 = 126 <= 128)
ROWS = GROUP * NB                 # 126 used partitions
QG = 8                            # groups per oct (two PSUM pages)
PAGES = QG // 4                   # PSUM pages per oct
QTOK = GROUP * QG                 # 112 tokens per oct
NQUAD = (TOK + QTOK - 1) // QTOK  # 10 (last oct ragged: 16 real tokens)

DVE_COPY_COLS = int(os.environ.get("BLOCKATTN_DVE_COPY", "768"))
ACT_SET = "natural_log_exp_and_others"

_CACHE = {}


def _groups(q):
    """[(g, t0, tg)] active groups of quad q (t0 = core-local token base)."""
    out = []
    for g in range(QG):
        t0 = q * QTOK + g * GROUP
        tg = min(GROUP, TOK - t0)
        if tg > 0:
            out.append((g, t0, tg))
    return out


def _patch_act_tables():
    """Make every activation func this kernel uses resolve to one table set
    (ACT_SET), so bacc emits a single ACT_TABLE_LOAD instead of thrashing
    between sets on every Ln/Exp/Square transition."""
    import concourse.bacc as bacc_mod
    import concourse.hw_specs as hw_specs
    from concourse import mybir

    if getattr(bacc_mod, "_blockattn_act_patch", False):
        return
    AF = mybir.ActivationFunctionType
    mine = {AF.Square, AF.Exp, AF.Ln, AF.Copy, AF.Identity}
    orig = hw_specs.get_activation_tables

    def patched(arch):
        t = dict(orig(arch))
        assert ACT_SET in t and mine <= t[ACT_SET], (ACT_SET, t.get(ACT_SET))
        return {
            name: (funcs if name == ACT_SET else funcs - mine)
            for name, funcs in t.items()
        }

    bacc_mod.get_activation_tables = patched
    bacc_mod._blockattn_act_patch = True


def build_nc():
    import concourse.bacc as bacc
    import concourse.tile as tile
    from concourse import mybir

    _patch_act_tables()

    f32 = mybir.dt.float32
    AF = mybir.ActivationFunctionType
    OP = mybir.AluOpType

    nc = bacc.Bacc("TRN2", target_bir_lowering=False, debug=False)

    vst_d = nc.dram_tensor("vstack", [NQUAD, ROWS, QG * D], f32,
                           kind="ExternalInput")
    w2b_d = nc.dram_tensor("w2b", [ROWS, D], f32, kind="ExternalInput")
    oh_d = nc.dram_tensor("onehot", [ROWS, GROUP], f32, kind="ExternalInput")
    ohT_d = nc.dram_tensor("onehotT", [GROUP, ROWS], f32, kind="ExternalInput")
    oh8_d = nc.dram_tensor("onehot8", [ROWS, QG * GROUP], f32, kind="ExternalInput")
    h_d = nc.dram_tensor("h", [TOK, D], f32, kind="ExternalOutput")

    vst = vst_d.ap()
    hout = h_d.ap()

    with tile.TileContext(nc) as tc:
        import contextlib
        ctx = contextlib.ExitStack()
        with ctx:
            consts = ctx.enter_context(tc.tile_pool(name="consts", bufs=1))
            vq_pool = ctx.enter_context(tc.tile_pool(name="vq", bufs=4))
            scr_pool = ctx.enter_context(tc.tile_pool(name="scr", bufs=2))
            stats_pool = ctx.enter_context(tc.tile_pool(name="stats", bufs=4))
            small_pool = ctx.enter_context(tc.tile_pool(name="small", bufs=3))
            hsb_pool = ctx.enter_context(tc.tile_pool(name="hsb", bufs=3))
            hpage_pool = ctx.enter_context(
                tc.tile_pool(name="hpage", bufs=3, space="PSUM"))
            zp_pool = ctx.enter_context(
                tc.tile_pool(name="zp", bufs=1, space="PSUM"))
            rzb_pool = ctx.enter_context(
                tc.tile_pool(name="rzb", bufs=1, space="PSUM"))

            w2b = consts.tile([ROWS, D], f32)
            nc.sync.dma_start(w2b[:], w2b_d.ap()[:])
            oh = consts.tile([ROWS, GROUP], f32)
            nc.sync.dma_start(oh[:], oh_d.ap()[:])
            ohT = consts.tile([GROUP, ROWS], f32)
            nc.sync.dma_start(ohT[:], ohT_d.ap()[:])
            oh8 = consts.tile([ROWS, QG * GROUP], f32)
            nc.sync.dma_start(oh8[:], oh8_d.ap()[:])
            zero_col = consts.tile([ROWS, 1], f32)
            nc.vector.memset(zero_col[:], 0.0)
            eps_col = consts.tile([ROWS, 1], f32)
            nc.vector.memset(eps_col[:], EPS)

            for q in range(NQUAD):
                groups = _groups(q)

                vq = vq_pool.tile([ROWS, QG * D], f32)
                stats = stats_pool.tile([ROWS, 2 * QG], f32)

                # ---- input DMA ----
                # (tail oct: only transfer the columns of active groups;
                #  first octs: chunked so stats start before the whole slab
                #  lands)
                n_full = sum(1 for _, _, tg in groups if tg == GROUP)
                n_chunks = 4 if q == 0 else (2 if q == 1 else 1)
                if n_chunks > 1:
                    cw = len(groups) * D // n_chunks
                    for ci in range(n_chunks):
                        nc.sync.dma_start(vq[:, ci * cw:(ci + 1) * cw],
                                          vst[q][:, ci * cw:(ci + 1) * cw])
                else:
                    used = len(groups) * D
                    nc.sync.dma_start(vq[:, 0:used], vst[q][:, 0:used])

                # ---- per-group stats (one full pass each on ACT and DVE) ----
                for g, t0, tg in groups:
                    gc = g * D
                    sq_scr = scr_pool.tile([ROWS, D], f32, tag="sq_scr")
                    nc.scalar.activation(
                        sq_scr[0:ROWS, :], vq[0:ROWS, gc:gc + D], AF.Square,
                        bias=zero_col[:], accum_out=stats[:, g:g + 1])
                    u_scr = scr_pool.tile([ROWS, D], f32, tag="u_scr")
                    nc.vector.scalar_tensor_tensor(
                        out=u_scr[0:ROWS, :], in0=vq[0:ROWS, gc:gc + D],
                        scalar=1.0, in1=w2b[0:ROWS, :],
                        op0=OP.mult, op1=OP.mult,
                        accum_out=stats[:, QG + g:QG + g + 1])

                # ---- softmax small ops on [126, 4] stats ----
                lnq = small_pool.tile([ROWS, QG], f32, tag="lnq")
                nc.scalar.activation(lnq[:], stats[:, 0:QG], AF.Ln,
                                     bias=eps_col[:], scale=1.0 / D)
                rq = small_pool.tile([ROWS, QG], f32, tag="rq")
                nc.scalar.activation(rq[:], lnq[:], AF.Exp,
                                     bias=zero_col[:], scale=-0.5)
                lg = small_pool.tile([ROWS, QG], f32, tag="lg")
                nc.vector.tensor_mul(lg[:], stats[:, QG:2 * QG], rq[:])
                e_sb = small_pool.tile([ROWS, QG], f32, tag="e_sb")
                nc.scalar.activation(e_sb[:], lg[:], AF.Exp, bias=zero_col[:])

                zp = zp_pool.tile([GROUP, QG], f32)
                nc.tensor.matmul(zp[:], lhsT=oh[:], rhs=e_sb[:],
                                 start=True, stop=True)
                rz = small_pool.tile([GROUP, QG], f32, tag="rz")
                nc.vector.reciprocal(rz[:], zp[:])
                rzb = rzb_pool.tile([ROWS, QG], f32)
                nc.tensor.matmul(rzb[:], lhsT=ohT[:], rhs=rz[:],
                                 start=True, stop=True)
                wcol = small_pool.tile([ROWS, QG], f32, tag="wcol")
                nc.vector.tensor_mul(wcol[:], e_sb[:], rzb[:])

                # ---- weighted sum via PE, 4 groups per PSUM page ----
                lhsTs = small_pool.tile([ROWS, QG * GROUP], f32, tag="lhsTs")
                active_pages = sorted({g // 4 for g, _, _ in groups})
                hpages = {pg: hpage_pool.tile([128, D], f32, tag="hpage",
                                              name="hpage")
                          for pg in active_pages}
                nc.vector.tensor_tensor(
                    out=lhsTs[:, :].rearrange("p (g j) -> p g j", g=QG),
                    in0=oh8[:, :].rearrange("p (g j) -> p g j", g=QG),
                    in1=wcol[:, :].unsqueeze(2).to_broadcast(
                        [ROWS, QG, GROUP]),
                    op=OP.mult)
                for g, t0, tg in groups:
                    gc = g * D
                    lw = lhsTs[:, g * GROUP:(g + 1) * GROUP]
                    pg = g // 4
                    col = 32 * (g % 4)
                    for hh in range(2):
                        nc.tensor.matmul(
                            hpages[pg][col:col + GROUP,
                                       512 * hh:512 * hh + 512],
                            lhsT=lw,
                            rhs=vq[0:ROWS, gc + 512 * hh:gc + 512 * hh + 512],
                            start=True, stop=True,
                            tile_position=(0, col))

                # ---- PSUM -> SBUF (split across DVE and ACT) -> HBM ----
                for pg in active_pages:
                    h_sb = hsb_pool.tile([128, D], f32, tag="h_sb")
                    nc.vector.tensor_copy(h_sb[:, 0:DVE_COPY_COLS],
                                          hpages[pg][:, 0:DVE_COPY_COLS])
                    nc.scalar.copy(h_sb[:, DVE_COPY_COLS:D],
                                   hpages[pg][:, DVE_COPY_COLS:D])
                    for g, t0, tg in groups:
                        if g // 4 != pg:
                            continue
                        nc.gpsimd.dma_start(hout[t0:t0 + tg, :],
                                            h_sb[32 * (g % 4):32 * (g % 4) + tg, :])

    nc.compile()
    return nc


def _host_inputs(blocks, partial_block, proj_w, norm_w):
    """Slice + interleave per-core inputs (host-side, numpy only)."""
    blocks = np.ascontiguousarray(blocks, dtype=np.float32).reshape(N, B * T, D)
    partial = np.ascontiguousarray(partial_block, dtype=np.float32).reshape(B * T, D)
    w2 = (np.asarray(proj_w, np.float32) * np.asarray(norm_w, np.float32))
    w2b = np.ascontiguousarray(np.broadcast_to(w2, (ROWS, D)), np.float32)
    oh = np.zeros((ROWS, GROUP), np.float32)
    for p in range(ROWS):
        oh[p, p % GROUP] = 1.0
    ohT = np.ascontiguousarray(oh.T)
    oh8 = np.ascontiguousarray(np.tile(oh, (1, QG)))

    pad_tok = NQUAD * QTOK  # 1064
    in_maps = []
    for c in range(N_CORES):
        s = slice(c * TOK, (c + 1) * TOK)
        av = np.zeros((NB, pad_tok, D), np.float32)
        av[:N, :TOK] = blocks[:, s, :]
        av[N, :TOK] = partial[s, :]
        # vstack[q, 14n+t', g*D+d] = av[n, q*56 + g*14 + t', d]
        vst = av.reshape(NB, NQUAD, QG, GROUP, D)
        vst = np.ascontiguousarray(vst.transpose(1, 0, 3, 2, 4))
        vst = vst.reshape(NQUAD, ROWS, QG * D)
        in_maps.append({
            "vstack": vst,
            "w2b": w2b,
            "onehot": oh,
            "onehotT": ohT,
            "onehot8": oh8,
        })
    return in_maps


def kernel(blocks, partial_block, proj_w, norm_w):
    from concourse.bass_utils import run_bass_kernel_spmd

    if "nc" not in _CACHE:
        _CACHE["nc"] = build_nc()
    nc = _CACHE["nc"]
    in_maps = _host_inputs(blocks, partial_block, proj_w, norm_w)
    res = run_bass_kernel_spmd(nc, in_maps, core_ids=list(range(N_CORES)))
    h = np.concatenate([res.results[c]["h"] for c in range(N_CORES)], axis=0)
    return h.reshape(B, T, D)



# revision 5
# speedup vs baseline: 1.2543x; 1.2543x over previous
"""BlockAttentionResidual Trainium2 kernel (v2: fp16, 32-token groups).

Math per token t over NB=9 blocks (8 full + 1 partial), D=1024:
    rq[n,t]   = (ssq[n,t]/D + eps)^(-1/2)
    logit     = (sum_d w2[d]*v[n,t,d]) * rq,   w2 = proj_w*norm_w
    w[n,t]    = softmax_n(logit)
    h[t,d]    = sum_n w[n,t]*v[n,t,d]

Sharding: B*T = 8192 tokens -> 1024 tokens/core on 8 cores.
Per core: 8 superquads (SQ) of 128 tokens = 4 groups x 32 tokens.
fp16 data path (host-side cast; harness tolerance is 2e-2, fp16 keeps
rel err ~1e-3), fp32 accumulation in PSUM / stats.

Layout per SQ (partition dim first):
  slabA[p = n*32+t', c = g*1024+d]  n in 0..3   [128, 4096]
  slabB: same for n in 4..7                      [128, 4096]
  slabP[p = g*32+t', d]  partial block           [128, 1024]

Stats: ssq via ACT Square+accum_out, dot via DVE stt+accum_out, into
stats[:, 0:9]=ssq / [:, 9:18]=dot (order: A g0-3, B g0-3, P).
Softmax over n via PE one-hot matmuls (Z accumulated in PSUM):
  Z[t',g] = ohA^T@eA + ohA^T@eB + (ohA*e8)^T@gsel ; rz = 1/Z
h accumulated UNNORMALIZED in PSUM (weights = raw e values):
  per (group, 512-col half): 3 accumulated matmuls
  (lhsT = ohA8*e masks for A/B; diag(e8) built from I128 for P).
Normalization by 1/Z happens in the PSUM->SBUF copy:
  rzcol[p = g*32+t'] = rz[t',g] (via rzsel matmul), then
  h_sb = hpage * rzcol (ACT Copy with scale AP / DVE tensor_scalar).
PSUM pages are fully packed (128 tokens/page) so the copy and the
output DMA are contiguous [128, 1024].
Three-stage software pipeline: stats(i) | softmax+mm(i-1) | copy+out(i-2).
"""

import os
import sys
import numpy as np

for _p in ("/opt/trn_rl_repo", "/root/.axon_site/_ro/trn_rl_repo"):
    if os.path.isdir(_p) and _p not in sys.path:
        sys.path.append(_p)

N_CORES = 8
N, B, T, D = 8, 4, 2048, 1024
EPS = 1e-6
TOK = (B * T) // N_CORES          # 1024 tokens per core
TPG = 32                          # tokens per group
NG = 4                            # groups per superquad
SQTOK = TPG * NG                  # 128 tokens per superquad
NSQ = TOK // SQTOK                # 8 superquads per core

# knobs for ACT/DVE balance
COPY_DVE = int(os.environ.get("BLOCKATTN_COPY_DVE", "128"))
SSQ_DVE = int(os.environ.get("BLOCKATTN_SSQ_DVE", "0"))  # of 9 ssq units -> DVE
ACT_SET = "natural_log_exp_and_others"

_CACHE = {}


def _patch_act_tables():
    """Make every activation func this kernel uses resolve to one table set
    (ACT_SET), so bacc emits a single ACT_TABLE_LOAD instead of thrashing
    between sets on every Ln/Exp/Square transition."""
    import concourse.bacc as bacc_mod
    import concourse.hw_specs as hw_specs
    from concourse import mybir

    if getattr(bacc_mod, "_blockattn_act_patch", False):
        return
    AF = mybir.ActivationFunctionType
    mine = {AF.Square, AF.Exp, AF.Ln, AF.Copy, AF.Identity}
    orig = hw_specs.get_activation_tables

    def patched(arch):
        t = dict(orig(arch))
        assert ACT_SET in t and mine <= t[ACT_SET], (ACT_SET, t.get(ACT_SET))
        return {
            name: (funcs if name == ACT_SET else funcs - mine)
            for name, funcs in t.items()
        }

    bacc_mod.get_activation_tables = patched
    bacc_mod._blockattn_act_patch = True


def build_nc():
    import concourse.bacc as bacc
    import concourse.tile as tile
    from concourse import mybir

    _patch_act_tables()

    f32 = mybir.dt.float32
    f16 = mybir.dt.float16
    AF = mybir.ActivationFunctionType
    OP = mybir.AluOpType

    nc = bacc.Bacc("TRN2", target_bir_lowering=False, debug=False)

    slabA_d = nc.dram_tensor("slabA", [NSQ, 128, NG * D], f16, kind="ExternalInput")
    slabB_d = nc.dram_tensor("slabB", [NSQ, 128, NG * D], f16, kind="ExternalInput")
    slabP_d = nc.dram_tensor("slabP", [NSQ, 128, D], f16, kind="ExternalInput")
    w2b_d = nc.dram_tensor("w2b", [128, D], f16, kind="ExternalInput")
    ohA_d = nc.dram_tensor("ohA", [128, TPG], f16, kind="ExternalInput")
    ohA8_d = nc.dram_tensor("ohA8", [128, 8 * TPG], f16, kind="ExternalInput")
    gsel_d = nc.dram_tensor("gsel", [128, NG], f16, kind="ExternalInput")
    ohAT_d = nc.dram_tensor("ohAT", [TPG, 128], f16, kind="ExternalInput")
    ones32_d = nc.dram_tensor("ones32", [TPG, 1], f16, kind="ExternalInput")
    ieye_d = nc.dram_tensor("ieye", [128, 128], f16, kind="ExternalInput")
    h_d = nc.dram_tensor("h", [TOK, D], f16, kind="ExternalOutput")

    vA = slabA_d.ap()
    vB = slabB_d.ap()
    vP = slabP_d.ap()
    hout = h_d.ap()

    with tile.TileContext(nc) as tc:
        import contextlib
        ctx = contextlib.ExitStack()
        with ctx:
            consts = ctx.enter_context(tc.tile_pool(name="consts", bufs=1))
            pA = ctx.enter_context(tc.tile_pool(name="pA", bufs=NSQ))
            pB = ctx.enter_context(tc.tile_pool(name="pB", bufs=NSQ))
            pP = ctx.enter_context(tc.tile_pool(name="pP", bufs=NSQ))
            stats_pool = ctx.enter_context(tc.tile_pool(name="stats", bufs=3))
            sm_pool = ctx.enter_context(tc.tile_pool(name="sm", bufs=2))
            hsb_pool = ctx.enter_context(tc.tile_pool(name="hsb", bufs=3))
            hpage_pool = ctx.enter_context(
                tc.tile_pool(name="hpage", bufs=3, space="PSUM"))
            z_pool = ctx.enter_context(
                tc.tile_pool(name="zp", bufs=1, space="PSUM"))
            rzb_pool = ctx.enter_context(
                tc.tile_pool(name="rzb", bufs=1, space="PSUM"))

            # ---- consts ----
            w2b = consts.tile([128, D], f16)
            nc.sync.dma_start(w2b[:], w2b_d.ap()[:])
            ohA = consts.tile([128, TPG], f16)
            nc.sync.dma_start(ohA[:], ohA_d.ap()[:])
            ohA8 = consts.tile([128, 8 * TPG], f16)
            nc.sync.dma_start(ohA8[:], ohA8_d.ap()[:])
            gsel = consts.tile([128, NG], f16)
            nc.sync.dma_start(gsel[:], gsel_d.ap()[:])
            ohAT = consts.tile([TPG, 128], f16)
            nc.sync.dma_start(ohAT[:], ohAT_d.ap()[:])
            ones32 = consts.tile([TPG, 1], f16)
            nc.sync.dma_start(ones32[:], ones32_d.ap()[:])
            ieye = consts.tile([128, 128], f16)
            nc.sync.dma_start(ieye[:], ieye_d.ap()[:])
            eps_col = consts.tile([128, 1], f32)
            nc.vector.memset(eps_col[:], EPS)
            zero_col = consts.tile([128, 1], f32)
            nc.vector.memset(zero_col[:], 0.0)
            # elementwise-output scratch (values never read; overwritten
            # in program order on each engine)
            scrA = consts.tile([128, D], f16)
            scrD = consts.tile([128, D], f16)

            # ---- input DMA: everything prefetched up-front ----
            slabA_t, slabB_t, slabP_t = [], [], []
            for sq in range(NSQ):
                ta = pA.tile([128, NG * D], f16, tag="slabA")
                tb = pB.tile([128, NG * D], f16, tag="slabB")
                tp = pP.tile([128, D], f16, tag="slabP")
                slabA_t.append(ta)
                slabB_t.append(tb)
                slabP_t.append(tp)
                nch = 4 if sq == 0 else (2 if sq == 1 else 1)
                cw = NG * D // nch
                for ci in range(nch):
                    sl = slice(ci * cw, (ci + 1) * cw)
                    nc.sync.dma_start(ta[:, sl], vA[sq][:, sl])
                for ci in range(nch):
                    sl = slice(ci * cw, (ci + 1) * cw)
                    nc.sync.dma_start(tb[:, sl], vB[sq][:, sl])
                nc.sync.dma_start(tp[:, :], vP[sq][:, :])

            state = {}

            def emit_stats(i):
                st = stats_pool.tile([128, 18], f32, tag="stats")
                state[i] = {"stats": st}
                # ssq units (9): ACT Square+accum (last SSQ_DVE of them on DVE)
                units = [(slabA_t[i], g * D, g) for g in range(NG)] \
                    + [(slabB_t[i], g * D, 4 + g) for g in range(NG)] \
                    + [(slabP_t[i], 0, 8)]
                for t, c0, sc in units[:9 - SSQ_DVE]:
                    nc.scalar.activation(
                        scrA[:, :], t[:, c0:c0 + D], AF.Square,
                        bias=zero_col[:], accum_out=st[:, sc:sc + 1])
                for t, c0, sc in units[9 - SSQ_DVE:]:
                    nc.vector.scalar_tensor_tensor(
                        out=scrD[:, :], in0=t[:, c0:c0 + D],
                        scalar=1.0, in1=t[:, c0:c0 + D],
                        op0=OP.mult, op1=OP.mult,
                        accum_out=st[:, sc:sc + 1])
                # dot units (9): DVE stt with w2b
                for t, c0, sc in units:
                    nc.vector.scalar_tensor_tensor(
                        out=scrD[:, :], in0=t[:, c0:c0 + D],
                        scalar=1.0, in1=w2b[:, :],
                        op0=OP.mult, op1=OP.mult,
                        accum_out=st[:, 9 + sc:10 + sc])

            def emit_softmax(i):
                st = state[i]["stats"]
                lnq = sm_pool.tile([128, 9], f32, tag="lnq")
                nc.scalar.activation(lnq[:], st[:, 0:9], AF.Ln,
                                     bias=eps_col[:], scale=1.0 / D)
                rq = sm_pool.tile([128, 9], f32, tag="rq")
                nc.scalar.activation(rq[:], lnq[:], AF.Exp,
                                     bias=zero_col[:], scale=-0.5)
                lg = sm_pool.tile([128, 9], f32, tag="lg")
                nc.vector.tensor_tensor(out=lg[:], in0=st[:, 9:18],
                                        in1=rq[:], op=OP.mult)
                e_all = sm_pool.tile([128, 9], f16, tag="e_all")
                nc.scalar.activation(e_all[:], lg[:], AF.Exp,
                                     bias=zero_col[:])
                e8m = sm_pool.tile([128, TPG], f16, tag="e8m")
                nc.vector.tensor_tensor(
                    out=e8m[:], in0=ohA[:],
                    in1=e_all[:, 8:9].to_broadcast([128, TPG]), op=OP.mult)

                zp = z_pool.tile([TPG, NG], f32)
                nc.tensor.matmul(zp[:], lhsT=ohA[:], rhs=e_all[:, 0:4],
                                 start=True, stop=False)
                nc.tensor.matmul(zp[:], lhsT=ohA[:], rhs=e_all[:, 4:8],
                                 start=False, stop=False)
                nc.tensor.matmul(zp[:], lhsT=e8m[:], rhs=gsel[:],
                                 start=False, stop=True)
                rz = sm_pool.tile([TPG, NG], f32, tag="rz")
                nc.vector.reciprocal(rz[:], zp[:])
                # rzcol[p = g*32+t'] = rz[t', g] via rzsel matmul
                rzsel = sm_pool.tile([TPG, 128], f16, tag="rzsel")
                nc.vector.tensor_tensor(
                    out=rzsel[:, :].rearrange("q (g j) -> q g j", g=NG),
                    in0=ohAT[:, :].rearrange("q (g j) -> q g j", g=NG),
                    in1=rz[:, :].unsqueeze(2).to_broadcast([TPG, NG, TPG]),
                    op=OP.mult)
                rzb = rzb_pool.tile([128, 1], f32)
                nc.tensor.matmul(rzb[:], lhsT=rzsel[:], rhs=ones32[:],
                                 start=True, stop=True)
                rzcol = sm_pool.tile([128, 1], f32, tag="rzcol")
                nc.vector.tensor_copy(rzcol[:], rzb[:])

                # unnormalized weight masks from raw e values
                lhsTAB = sm_pool.tile([128, 8 * TPG], f16, tag="lhsTAB")
                nc.vector.tensor_tensor(
                    out=lhsTAB[:, :].rearrange("p (g j) -> p g j", g=8),
                    in0=ohA8[:, :].rearrange("p (g j) -> p g j", g=8),
                    in1=e_all[:, 0:8].unsqueeze(2).to_broadcast([128, 8, TPG]),
                    op=OP.mult)
                lhsTPd = sm_pool.tile([128, 128], f16, tag="lhsTPd")
                nc.vector.tensor_tensor(
                    out=lhsTPd[:], in0=ieye[:],
                    in1=e_all[:, 8:9].to_broadcast([128, 128]), op=OP.mult)

                hpage = hpage_pool.tile([128, D], f32, tag="hpage",
                                        name="hpage")
                for g in range(NG):
                    col = g * TPG
                    for hh in range(2):
                        osl = slice(512 * hh, 512 * hh + 512)
                        nc.tensor.matmul(
                            hpage[col:col + TPG, osl],
                            lhsT=lhsTAB[:, col:col + TPG],
                            rhs=slabA_t[i][:, g * D + 512 * hh:
                                           g * D + 512 * hh + 512],
                            start=True, stop=False, tile_position=(0, col))
                        nc.tensor.matmul(
                            hpage[col:col + TPG, osl],
                            lhsT=lhsTAB[:, 128 + col:128 + col + TPG],
                            rhs=slabB_t[i][:, g * D + 512 * hh:
                                           g * D + 512 * hh + 512],
                            start=False, stop=False, tile_position=(0, col))
                        nc.tensor.matmul(
                            hpage[col:col + TPG, osl],
                            lhsT=lhsTPd[:, col:col + TPG],
                            rhs=slabP_t[i][:, osl],
                            start=False, stop=True, tile_position=(0, col))
                state[i]["hpage"] = hpage
                state[i]["rzcol"] = rzcol

            def emit_out(i):
                hpage = state[i]["hpage"]
                rzcol = state[i]["rzcol"]
                h_sb = hsb_pool.tile([128, D], f16, tag="h_sb")
                if COPY_DVE > 0:
                    nc.vector.tensor_scalar_mul(
                        h_sb[:, 0:COPY_DVE], hpage[:, 0:COPY_DVE], rzcol[:])
                if COPY_DVE < D:
                    nc.scalar.activation(h_sb[:, COPY_DVE:D],
                                         hpage[:, COPY_DVE:D],
                                         AF.Copy, scale=rzcol[:])
                nc.gpsimd.dma_start(hout[i * SQTOK:(i + 1) * SQTOK, :],
                                    h_sb[:, :])
                del state[i]

            for i in range(NSQ + 2):
                if i < NSQ:
                    emit_stats(i)
                if 1 <= i <= NSQ:
                    emit_softmax(i - 1)
                if i >= 2:
                    emit_out(i - 2)

    nc.compile()
    return nc


def _host_inputs(blocks, partial_block, proj_w, norm_w):
    """Slice + rearrange per-core inputs (host-side, numpy only)."""
    blocks16 = np.asarray(blocks, np.float16).reshape(N, B * T, D)
    partial16 = np.asarray(partial_block, np.float16).reshape(B * T, D)
    w2 = (np.asarray(proj_w, np.float32)
          * np.asarray(norm_w, np.float32)).astype(np.float16)
    w2b = np.ascontiguousarray(np.broadcast_to(w2, (128, D)))
    p = np.arange(128)
    ohA = (p[:, None] % TPG == np.arange(TPG)[None, :]).astype(np.float16)
    ohA8 = np.ascontiguousarray(np.tile(ohA, (1, 8)))
    gsel = (p[:, None] // TPG == np.arange(NG)[None, :]).astype(np.float16)
    ohAT = np.ascontiguousarray(ohA.T)
    ones32 = np.ones((TPG, 1), np.float16)
    ieye = np.eye(128, dtype=np.float16)

    in_maps = []
    for c in range(N_CORES):
        s = slice(c * TOK, (c + 1) * TOK)
        # slabA[sq, n*32+t', g*1024+d] = blocks[n, sq*128+g*32+t', d]
        ba = blocks16[0:4, s].reshape(4, NSQ, NG, TPG, D)
        slabA = np.ascontiguousarray(
            ba.transpose(1, 0, 3, 2, 4)).reshape(NSQ, 128, NG * D)
        bb = blocks16[4:8, s].reshape(4, NSQ, NG, TPG, D)
        slabB = np.ascontiguousarray(
            bb.transpose(1, 0, 3, 2, 4)).reshape(NSQ, 128, NG * D)
        slabP = np.ascontiguousarray(partial16[s].reshape(NSQ, 128, D))
        in_maps.append({
            "slabA": slabA,
            "slabB": slabB,
            "slabP": slabP,
            "w2b": w2b,
            "ohA": ohA,
            "ohA8": ohA8,
            "gsel": gsel,
            "ohAT": ohAT,
            "ones32": ones32,
            "ieye": ieye,
        })
    return in_maps


def kernel(blocks, partial_block, proj_w, norm_w):
    from concourse.bass_utils import run_bass_kernel_spmd

    if "nc" not in _CACHE:
        _CACHE["nc"] = build_nc()
    nc = _CACHE["nc"]
    in_maps = _host_inputs(blocks, partial_block, proj_w, norm_w)
    res = run_bass_kernel_spmd(nc, in_maps, core_ids=list(range(N_CORES)))
    h = np.concatenate([np.asarray(res.results[c]["h"])
                        for c in range(N_CORES)], axis=0)
    return h.astype(np.float32).reshape(B, T, D)


# revision 9
# speedup vs baseline: 1.4782x; 1.1785x over previous
"""BlockAttentionResidual Trainium2 kernel (v2: fp16, 32-token groups).

Math per token t over NB=9 blocks (8 full + 1 partial), D=1024:
    rq[n,t]   = (ssq[n,t]/D + eps)^(-1/2)
    logit     = (sum_d w2[d]*v[n,t,d]) * rq,   w2 = proj_w*norm_w
    w[n,t]    = softmax_n(logit)
    h[t,d]    = sum_n w[n,t]*v[n,t,d]

Sharding: B*T = 8192 tokens -> 1024 tokens/core on 8 cores.
Per core: 8 superquads (SQ) of 128 tokens = 4 groups x 32 tokens.
fp16 data path (host-side cast; harness tolerance is 2e-2, fp16 keeps
rel err ~1e-3), fp32 accumulation in PSUM / stats.

Layout per SQ (partition dim first):
  slabA[p = n*32+t', c = g*1024+d]  n in 0..3   [128, 4096]
  slabB: same for n in 4..7                      [128, 4096]
  slabP[p = g*32+t', d]  partial block           [128, 1024]

Stats: ssq via ACT Square+accum_out, dot via DVE stt+accum_out, into
stats[:, 0:9]=ssq / [:, 9:18]=dot (order: A g0-3, B g0-3, P).
Softmax over n via PE one-hot matmuls (Z accumulated in PSUM):
  Z[t',g] = ohA^T@eA + ohA^T@eB + (ohA*e8)^T@gsel ; rz = 1/Z
h accumulated UNNORMALIZED in PSUM (weights = raw e values):
  per (group, 512-col half): 3 accumulated matmuls
  (lhsT = ohA8*e masks for A/B; diag(e8) built from I128 for P).
Normalization by 1/Z happens in the PSUM->SBUF copy:
  rzcol[p = g*32+t'] = rz[t',g] (via rzsel matmul), then
  h_sb = hpage * rzcol (ACT Copy with scale AP / DVE tensor_scalar).
PSUM pages are fully packed (128 tokens/page) so the copy and the
output DMA are contiguous [128, 1024].
Three-stage software pipeline: stats(i) | softmax+mm(i-1) | copy+out(i-2).
"""

import os
import sys
import numpy as np

for _p in ("/opt/trn_rl_repo", "/root/.axon_site/_ro/trn_rl_repo"):
    if os.path.isdir(_p) and _p not in sys.path:
        sys.path.append(_p)

N_CORES = 8
N, B, T, D = 8, 4, 2048, 1024
EPS = 1e-6
TOK = (B * T) // N_CORES          # 1024 tokens per core
TPG = 32                          # tokens per group
NG = 4                            # groups per superquad
SQTOK = TPG * NG                  # 128 tokens per superquad
NSQ = TOK // SQTOK                # 8 superquads per core

# knobs for ACT/DVE balance
COPY_DVE = int(os.environ.get("BLOCKATTN_COPY_DVE", "128"))
SSQ_DVE = int(os.environ.get("BLOCKATTN_SSQ_DVE", "0"))  # of 9 ssq units -> DVE
DTYPE = os.environ.get("BLOCKATTN_DTYPE", "bf16")  # bf16 | fp16
ACT_SET = "natural_log_exp_and_others"

_CACHE = {}


def _patch_act_tables():
    """Make every activation func this kernel uses resolve to one table set
    (ACT_SET), so bacc emits a single ACT_TABLE_LOAD instead of thrashing
    between sets on every Ln/Exp/Square transition."""
    import concourse.bacc as bacc_mod
    import concourse.hw_specs as hw_specs
    from concourse import mybir

    if getattr(bacc_mod, "_blockattn_act_patch", False):
        return
    AF = mybir.ActivationFunctionType
    mine = {AF.Square, AF.Exp, AF.Ln, AF.Copy, AF.Identity}
    orig = hw_specs.get_activation_tables

    def patched(arch):
        t = dict(orig(arch))
        assert ACT_SET in t and mine <= t[ACT_SET], (ACT_SET, t.get(ACT_SET))
        return {
            name: (funcs if name == ACT_SET else funcs - mine)
            for name, funcs in t.items()
        }

    bacc_mod.get_activation_tables = patched
    bacc_mod._blockattn_act_patch = True


def build_nc():
    import concourse.bacc as bacc
    import concourse.tile as tile
    from concourse import mybir

    _patch_act_tables()

    f32 = mybir.dt.float32
    f16 = mybir.dt.bfloat16 if DTYPE == "bf16" else mybir.dt.float16
    AF = mybir.ActivationFunctionType
    OP = mybir.AluOpType

    nc = bacc.Bacc("TRN2", target_bir_lowering=False, debug=False)

    slabA_d = nc.dram_tensor("slabA", [NSQ, 128, NG * D], f16, kind="ExternalInput")
    slabB_d = nc.dram_tensor("slabB", [NSQ, 128, NG * D], f16, kind="ExternalInput")
    slabP_d = nc.dram_tensor("slabP", [NSQ, 128, D], f16, kind="ExternalInput")
    w2b_d = nc.dram_tensor("w2b", [128, D], f16, kind="ExternalInput")
    ohA_d = nc.dram_tensor("ohA", [128, TPG], f16, kind="ExternalInput")
    ohA8_d = nc.dram_tensor("ohA8", [128, 8 * TPG], f16, kind="ExternalInput")
    gsel_d = nc.dram_tensor("gsel", [128, NG], f16, kind="ExternalInput")
    ohAT_d = nc.dram_tensor("ohAT", [TPG, 128], f16, kind="ExternalInput")
    ones32_d = nc.dram_tensor("ones32", [TPG, 1], f16, kind="ExternalInput")
    ieye_d = nc.dram_tensor("ieye", [128, 128], f16, kind="ExternalInput")
    h_d = nc.dram_tensor("h", [TOK, D], f16, kind="ExternalOutput")

    vA = slabA_d.ap()
    vB = slabB_d.ap()
    vP = slabP_d.ap()
    hout = h_d.ap()

    with tile.TileContext(nc) as tc:
        import contextlib
        ctx = contextlib.ExitStack()
        with ctx:
            consts = ctx.enter_context(tc.tile_pool(name="consts", bufs=1))
            pA = ctx.enter_context(tc.tile_pool(name="pA", bufs=NSQ))
            pB = ctx.enter_context(tc.tile_pool(name="pB", bufs=NSQ))
            pP = ctx.enter_context(tc.tile_pool(name="pP", bufs=NSQ))
            stats_pool = ctx.enter_context(tc.tile_pool(name="stats", bufs=3))
            sm_pool = ctx.enter_context(tc.tile_pool(name="sm", bufs=2))
            hsb_pool = ctx.enter_context(tc.tile_pool(name="hsb", bufs=3))
            hpage_pool = ctx.enter_context(
                tc.tile_pool(name="hpage", bufs=3, space="PSUM"))
            z_pool = ctx.enter_context(
                tc.tile_pool(name="zp", bufs=1, space="PSUM"))
            rzb_pool = ctx.enter_context(
                tc.tile_pool(name="rzb", bufs=1, space="PSUM"))

            # ---- consts ----
            w2b = consts.tile([128, D], f16)
            nc.sync.dma_start(w2b[:], w2b_d.ap()[:])
            ohA = consts.tile([128, TPG], f16)
            nc.sync.dma_start(ohA[:], ohA_d.ap()[:])
            ohA8 = consts.tile([128, 8 * TPG], f16)
            nc.sync.dma_start(ohA8[:], ohA8_d.ap()[:])
            gsel = consts.tile([128, NG], f16)
            nc.sync.dma_start(gsel[:], gsel_d.ap()[:])
            ohAT = consts.tile([TPG, 128], f16)
            nc.sync.dma_start(ohAT[:], ohAT_d.ap()[:])
            ones32 = consts.tile([TPG, 1], f16)
            nc.sync.dma_start(ones32[:], ones32_d.ap()[:])
            ieye = consts.tile([128, 128], f16)
            nc.sync.dma_start(ieye[:], ieye_d.ap()[:])
            eps_col = consts.tile([128, 1], f32)
            nc.vector.memset(eps_col[:], EPS)
            zero_col = consts.tile([128, 1], f32)
            nc.vector.memset(zero_col[:], 0.0)
            # elementwise-output scratch (values never read; overwritten
            # in program order on each engine)
            scrA = consts.tile([128, D], f16)
            scrD = consts.tile([128, D], f16)

            # ---- input DMA: everything prefetched up-front ----
            slabA_t, slabB_t, slabP_t = [], [], []
            for sq in range(NSQ):
                ta = pA.tile([128, NG * D], f16, tag="slabA")
                tb = pB.tile([128, NG * D], f16, tag="slabB")
                tp = pP.tile([128, D], f16, tag="slabP")
                slabA_t.append(ta)
                slabB_t.append(tb)
                slabP_t.append(tp)
                nch = 4 if sq == 0 else (2 if sq == 1 else 1)
                cw = NG * D // nch
                for ci in range(nch):
                    sl = slice(ci * cw, (ci + 1) * cw)
                    nc.sync.dma_start(ta[:, sl], vA[sq][:, sl])
                for ci in range(nch):
                    sl = slice(ci * cw, (ci + 1) * cw)
                    nc.sync.dma_start(tb[:, sl], vB[sq][:, sl])
                nc.sync.dma_start(tp[:, :], vP[sq][:, :])

            state = {}

            def emit_stats(i):
                st = stats_pool.tile([128, 18], f32, tag="stats")
                state[i] = {"stats": st}
                # ssq units (9): ACT Square+accum (last SSQ_DVE of them on DVE)
                units = [(slabA_t[i], g * D, g) for g in range(NG)] \
                    + [(slabB_t[i], g * D, 4 + g) for g in range(NG)] \
                    + [(slabP_t[i], 0, 8)]
                for t, c0, sc in units[:9 - SSQ_DVE]:
                    nc.scalar.activation(
                        scrA[:, :], t[:, c0:c0 + D], AF.Square,
                        bias=zero_col[:], accum_out=st[:, sc:sc + 1])
                for t, c0, sc in units[9 - SSQ_DVE:]:
                    nc.vector.scalar_tensor_tensor(
                        out=scrD[:, :], in0=t[:, c0:c0 + D],
                        scalar=1.0, in1=t[:, c0:c0 + D],
                        op0=OP.mult, op1=OP.mult,
                        accum_out=st[:, sc:sc + 1])
                # dot units (9): DVE stt with w2b
                for t, c0, sc in units:
                    nc.vector.scalar_tensor_tensor(
                        out=scrD[:, :], in0=t[:, c0:c0 + D],
                        scalar=1.0, in1=w2b[:, :],
                        op0=OP.mult, op1=OP.mult,
                        accum_out=st[:, 9 + sc:10 + sc])

            def emit_softmax(i):
                st = state[i]["stats"]
                lnq = sm_pool.tile([128, 9], f32, tag="lnq")
                nc.scalar.activation(lnq[:], st[:, 0:9], AF.Ln,
                                     bias=eps_col[:], scale=1.0 / D)
                rq = sm_pool.tile([128, 9], f32, tag="rq")
                nc.scalar.activation(rq[:], lnq[:], AF.Exp,
                                     bias=zero_col[:], scale=-0.5)
                lg = sm_pool.tile([128, 9], f32, tag="lg")
                nc.vector.tensor_tensor(out=lg[:], in0=st[:, 9:18],
                                        in1=rq[:], op=OP.mult)
                e_all = sm_pool.tile([128, 9], f16, tag="e_all")
                nc.scalar.activation(e_all[:], lg[:], AF.Exp,
                                     bias=zero_col[:])
                e8m = sm_pool.tile([128, TPG], f16, tag="e8m")
                nc.vector.tensor_tensor(
                    out=e8m[:], in0=ohA[:],
                    in1=e_all[:, 8:9].to_broadcast([128, TPG]), op=OP.mult)

                zp = z_pool.tile([TPG, NG], f32)
                nc.tensor.matmul(zp[:], lhsT=ohA[:], rhs=e_all[:, 0:4],
                                 start=True, stop=False)
                nc.tensor.matmul(zp[:], lhsT=ohA[:], rhs=e_all[:, 4:8],
                                 start=False, stop=False)
                nc.tensor.matmul(zp[:], lhsT=e8m[:], rhs=gsel[:],
                                 start=False, stop=True)
                rz = sm_pool.tile([TPG, NG], f32, tag="rz")
                nc.vector.reciprocal(rz[:], zp[:])
                # rzcol[p = g*32+t'] = rz[t', g] via rzsel matmul
                rzsel = sm_pool.tile([TPG, 128], f16, tag="rzsel")
                nc.vector.tensor_tensor(
                    out=rzsel[:, :].rearrange("q (g j) -> q g j", g=NG),
                    in0=ohAT[:, :].rearrange("q (g j) -> q g j", g=NG),
                    in1=rz[:, :].unsqueeze(2).to_broadcast([TPG, NG, TPG]),
                    op=OP.mult)
                rzb = rzb_pool.tile([128, 1], f32)
                nc.tensor.matmul(rzb[:], lhsT=rzsel[:], rhs=ones32[:],
                                 start=True, stop=True)
                rzcol = sm_pool.tile([128, 1], f32, tag="rzcol")
                nc.vector.tensor_copy(rzcol[:], rzb[:])

                # unnormalized weight masks from raw e values
                lhsTAB = sm_pool.tile([128, 8 * TPG], f16, tag="lhsTAB")
                nc.vector.tensor_tensor(
                    out=lhsTAB[:, :].rearrange("p (g j) -> p g j", g=8),
                    in0=ohA8[:, :].rearrange("p (g j) -> p g j", g=8),
                    in1=e_all[:, 0:8].unsqueeze(2).to_broadcast([128, 8, TPG]),
                    op=OP.mult)
                lhsTPd = sm_pool.tile([128, 128], f16, tag="lhsTPd")
                nc.vector.tensor_tensor(
                    out=lhsTPd[:], in0=ieye[:],
                    in1=e_all[:, 8:9].to_broadcast([128, 128]), op=OP.mult)

                hpage = hpage_pool.tile([128, D], f32, tag="hpage",
                                        name="hpage")
                for g in range(NG):
                    col = g * TPG
                    for hh in range(2):
                        osl = slice(512 * hh, 512 * hh + 512)
                        nc.tensor.matmul(
                            hpage[col:col + TPG, osl],
                            lhsT=lhsTAB[:, col:col + TPG],
                            rhs=slabA_t[i][:, g * D + 512 * hh:
                                           g * D + 512 * hh + 512],
                            start=True, stop=False, tile_position=(0, col))
                        nc.tensor.matmul(
                            hpage[col:col + TPG, osl],
                            lhsT=lhsTAB[:, 128 + col:128 + col + TPG],
                            rhs=slabB_t[i][:, g * D + 512 * hh:
                                           g * D + 512 * hh + 512],
                            start=False, stop=False, tile_position=(0, col))
                        nc.tensor.matmul(
                            hpage[col:col + TPG, osl],
                            lhsT=lhsTPd[:, col:col + TPG],
                            rhs=slabP_t[i][:, osl],
                            start=False, stop=True, tile_position=(0, col))
                state[i]["hpage"] = hpage
                state[i]["rzcol"] = rzcol

            def emit_out(i):
                hpage = state[i]["hpage"]
                rzcol = state[i]["rzcol"]
                h_sb = hsb_pool.tile([128, D], f16, tag="h_sb")
                if COPY_DVE > 0:
                    nc.vector.tensor_scalar_mul(
                        h_sb[:, 0:COPY_DVE], hpage[:, 0:COPY_DVE], rzcol[:])
                if COPY_DVE < D:
                    nc.scalar.activation(h_sb[:, COPY_DVE:D],
                                         hpage[:, COPY_DVE:D],
                                         AF.Copy, scale=rzcol[:])
                nc.gpsimd.dma_start(hout[i * SQTOK:(i + 1) * SQTOK, :],
                                    h_sb[:, :])
                del state[i]

            for i in range(NSQ + 2):
                if i < NSQ:
                    emit_stats(i)
                if 1 <= i <= NSQ:
                    emit_softmax(i - 1)
                if i >= 2:
                    emit_out(i - 2)

    nc.compile()
    return nc


def _host_inputs(blocks, partial_block, proj_w, norm_w):
    """Slice + rearrange per-core inputs (host-side, numpy only)."""
    if DTYPE == "bf16":
        import ml_dtypes
        npdt = np.dtype(ml_dtypes.bfloat16)
    else:
        npdt = np.dtype(np.float16)
    blocks16 = np.asarray(blocks, np.float32).astype(npdt).reshape(N, B * T, D)
    partial16 = np.asarray(partial_block, np.float32).astype(npdt).reshape(
        B * T, D)
    w2 = (np.asarray(proj_w, np.float32)
          * np.asarray(norm_w, np.float32)).astype(npdt)
    w2b = np.ascontiguousarray(np.broadcast_to(w2, (128, D)))
    p = np.arange(128)
    ohA = (p[:, None] % TPG == np.arange(TPG)[None, :]).astype(npdt)
    ohA8 = np.ascontiguousarray(np.tile(ohA, (1, 8)))
    gsel = (p[:, None] // TPG == np.arange(NG)[None, :]).astype(npdt)
    ohAT = np.ascontiguousarray(ohA.T)
    ones32 = np.ones((TPG, 1), npdt)
    ieye = np.eye(128).astype(npdt)

    in_maps = []
    for c in range(N_CORES):
        s = slice(c * TOK, (c + 1) * TOK)
        # slabA[sq, n*32+t', g*1024+d] = blocks[n, sq*128+g*32+t', d]
        ba = blocks16[0:4, s].reshape(4, NSQ, NG, TPG, D)
        slabA = np.ascontiguousarray(
            ba.transpose(1, 0, 3, 2, 4)).reshape(NSQ, 128, NG * D)
        bb = blocks16[4:8, s].reshape(4, NSQ, NG, TPG, D)
        slabB = np.ascontiguousarray(
            bb.transpose(1, 0, 3, 2, 4)).reshape(NSQ, 128, NG * D)
        slabP = np.ascontiguousarray(partial16[s].reshape(NSQ, 128, D))
        in_maps.append({
            "slabA": slabA,
            "slabB": slabB,
            "slabP": slabP,
            "w2b": w2b,
            "ohA": ohA,
            "ohA8": ohA8,
            "gsel": gsel,
            "ohAT": ohAT,
            "ones32": ones32,
            "ieye": ieye,
        })
    return in_maps


def kernel(blocks, partial_block, proj_w, norm_w):
    from concourse.bass_utils import run_bass_kernel_spmd

    if "nc" not in _CACHE:
        _CACHE["nc"] = build_nc()
    nc = _CACHE["nc"]
    in_maps = _host_inputs(blocks, partial_block, proj_w, norm_w)
    res = run_bass_kernel_spmd(nc, in_maps, core_ids=list(range(N_CORES)))
    h = np.concatenate([np.asarray(res.results[c]["h"])
                        for c in range(N_CORES)], axis=0)
    return h.astype(np.float32).reshape(B, T, D)
